# revision 26
# baseline (speedup 1.0000x reference)
"""Bass/Trainium2 kernel for a 6-layer dense transformer LM (BigramLanguageModel).

Sharding (8 cores): core c = (batch b = c//2, seq-half s = c%2).
Each core owns 512 tokens of one batch: runs the full 6-layer transformer on its
tokens, exchanging per-layer K/V with its pair core via pairwise AllGathers
(replica groups [[0,1],[2,3],[4,5],[6,7]]), then computes logits for its tokens
over the FULL vocab. Output is assembled on the host.

Device-side layout choices:
  - Activations are feature-major [D(6x128 partition chunks), T2=512(free)], so
    every projection uses the natural weight layout as matmul lhsT and produces
    feature-major output with zero transposes anywhere.
  - Attention computes S^T[k,q] = K-slices^T @ Q so softmax runs along the free
    dim; V is computed token-major so it is directly the PV lhsT; a built-in
    ones-column in V yields the softmax denominator in the same matmul.
  - Softmax skips max-subtraction (|scores*scale| < ~3 for this model family);
    1/x and 1/sqrt(x) are computed as exp(-ln(x)) / exp(-0.5 ln(x)) on the ACT
    engine (fast, and keeps a single activation-table set resident).
  - Matmuls in bf16; residual stream fp32; LN statistics via fp32r PE matmuls.
"""

import os
import sys

for _p in ("/opt/trn_rl_repo", "/root/.axon_site/_ro/trn_rl_repo"):
    if os.path.isdir(_p) and _p not in sys.path:
        sys.path.insert(0, _p)

import numpy as np
import ml_dtypes

import concourse.bass as bass
import concourse.mybir as mybir
import concourse.tile as tile
from concourse import bacc
from concourse import bass_utils

F32 = mybir.dt.float32
F32R = mybir.dt.float32r
BF16 = mybir.dt.bfloat16
AF = mybir.ActivationFunctionType
OP = mybir.AluOpType

L = 6
D = 768
H = 12
HD = 64
FF = 3072
V = 32000
VP = 32256  # padded vocab: 63 slices of 512
T = 1024
T2 = 512
B = 4
ND = D // 128   # 6 feature chunks
NF = FF // 128  # 24 ff chunks
NT = T2 // 128  # 4 own-token chunks
NK = T // 128   # 8 global key chunks
NV = VP // 512  # 63 vocab slices
SCALE = HD ** -0.5
EPS = 1e-5

# param columns in the packed per-layer param tile [128, 60]
P_LN1S, P_LN1B, P_LN2S, P_LN2B, P_BO, P_B2, P_B1 = 0, 6, 12, 18, 24, 30, 36
NPRM = 60

_BUILT = {}


def _act_single_set_tables(orig_get_tables):
    """All ACT funcs used here (Ln, Exp, Relu, Identity, Copy) live together in
    the `natural_log_exp_and_others` set; the default per-func set choice picks
    the first matching set and thrashes ACT_TABLE_LOADs (50 loads, ~64us, all
    serial on ACT). Strip those funcs from every other set so the load-insertion
    fixpoint has exactly one choice. Set ids (dict order) are preserved."""
    def patched(arch):
        t = dict(orig_get_tables(arch))
        keep = t["natural_log_exp_and_others"]
        return {
            name: (funcs if name == "natural_log_exp_and_others" else funcs - keep)
            for name, funcs in t.items()
        }
    return patched


def _build(nlayers=L):
    nc = bacc.Bacc("TRN2", target_bir_lowering=False, debug=False)

    h0_d = nc.dram_tensor("h0", [128, ND, T2], F32, kind="ExternalInput")
    msk_d = nc.dram_tensor("masks", [NK, 128, 128], BF16, kind="ExternalInput")
    wq_d = nc.dram_tensor("wq_t", [nlayers, ND, 128, ND, 128], BF16, kind="ExternalInput")
    wk_d = nc.dram_tensor("wk_t", [nlayers, ND, 128, ND, 128], BF16, kind="ExternalInput")
    wo_d = nc.dram_tensor("wo_t", [nlayers, ND, 128, ND, 128], BF16, kind="ExternalInput")
    wv_d = nc.dram_tensor("wv_t", [nlayers, 128, ND, D], BF16, kind="ExternalInput")
    w1_d = nc.dram_tensor("w1_t", [nlayers, NF, 128, ND, 128], BF16, kind="ExternalInput")
    w2_d = nc.dram_tensor("w2_t", [nlayers, ND, 128, NF, 128], BF16, kind="ExternalInput")
    wh_d = nc.dram_tensor("wh_t", [NV, 128, ND, 512], BF16, kind="ExternalInput")
    prm_d = nc.dram_tensor("prm", [nlayers, 128, NPRM], F32, kind="ExternalInput")
    lnf_d = nc.dram_tensor("lnf", [128, 2 * ND], F32, kind="ExternalInput")
    selA_d = nc.dram_tensor("selA", [6, ND, 128], F32, kind="ExternalInput")
    selB_d = nc.dram_tensor("selB", [6, ND, 128], F32, kind="ExternalInput")
    out_d = nc.dram_tensor("logits", [T2, VP], BF16, kind="ExternalOutput")

    rg = [[0, 1], [2, 3], [4, 5], [6, 7]]

    with tile.TileContext(nc) as tc:
        with tc.tile_pool(name="pers", bufs=1) as pers, \
             tc.tile_pool(name="sb", bufs=1) as sb, \
             tc.tile_pool(name="w", bufs=1) as wp, \
             tc.tile_pool(name="ps", bufs=1, space="PSUM") as ps, \
             tc.tile_pool(name="dram", bufs=1, space="DRAM") as dram:

            # ---------------- persistent tiles ----------------
            h = [pers.tile([128, T2], F32, name=f"h{m}") for m in range(ND)]
            for m in range(ND):
                nc.sync.dma_start(h[m][:], h0_d[:, m, :])
            msk = pers.tile([128, NK, 128], BF16)
            nc.sync.dma_start(msk[:], msk_d[:].rearrange("c p t -> p c t"))

            ones_f = pers.tile([128, 1], F32)
            nc.vector.memset(ones_f[:], 1.0)
            ones_r = pers.tile([128, 1], F32R)          # LN sum lhsT [K=128, M=1]
            nc.vector.tensor_copy(out=ones_r[:], in_=ones_f[:])
            onesM_f = pers.tile([1, 128], F32)
            nc.vector.memset(onesM_f[:], 1.0)
            onesM_r = pers.tile([1, 128], F32R)         # bcast lhsT [K=1, M<=128]
            nc.vector.tensor_copy(out=onesM_r[:], in_=onesM_f[:])

            lnf_sb = pers.tile([128, 2 * ND], F32)
            nc.sync.dma_start(lnf_sb[:], lnf_d[:])
            selA_r = pers.tile([6, ND, 128], F32R)
            selB_r = pers.tile([6, ND, 128], F32R)
            for _sd, _sr in ((selA_d, selA_r), (selB_d, selB_r)):
                sel_f = sb.tile([6, ND, 128], F32, tag="self", bufs=1, name=f"self_{_sd.name}")
                nc.sync.dma_start(sel_f[:], _sd[:])
                nc.vector.tensor_copy(out=_sr[:], in_=sel_f[:])

            import itertools
            _ln_ctr = itertools.count()

            # ---------------- helpers ----------------
            def layer_norm(src, s_ap, b_ap, tag="a"):
                """src: list of ND [128, T2] fp32 tiles -> list of ND bf16 tiles.
                s_ap/b_ap: [128, ND] fp32 scale/bias tiles.
                rstd = exp(-0.5*ln(var+eps)) keeps the serial chain short."""
                # stats borrow the attention po-tag banks (never live at the
                # same time: LN stats need the full residual, which needs all
                # attention outputs)
                s1 = ps.tile([1, T2], F32, tag="po", bufs=2)
                s2 = ps.tile([1, T2], F32, tag="po", bufs=2)
                for m in range(ND):
                    h_r = sb.tile([128, T2], F32R, tag="h_r", bufs=2)
                    nc.vector.tensor_copy(out=h_r[:], in_=src[m][:])
                    hsq = sb.tile([128, T2], F32R, tag="hsq", bufs=2)
                    nc.vector.tensor_mul(out=hsq[:], in0=src[m][:], in1=src[m][:])
                    nc.tensor.matmul(s1[:], ones_r[:], h_r[:], start=(m == 0), stop=(m == ND - 1))
                    nc.tensor.matmul(s2[:], ones_r[:], hsq[:], start=(m == 0), stop=(m == ND - 1))
                # u = (s2 + D*eps) - s1^2/D;  var+eps = u/D (Ln's free affine
                # applies the 1/D scale; Square's applies the 1/D inside)
                ssq = sb.tile([1, T2], F32, tag="lnstat", bufs=4)
                nc.scalar.activation(ssq[:], s1[:], AF.Square, scale=float(D) ** -0.5)
                u = sb.tile([1, T2], F32, tag="lnstat", bufs=4)
                nc.vector.scalar_tensor_tensor(
                    out=u[:], in0=s2[:], scalar=float(D * EPS), in1=ssq[:],
                    op0=OP.add, op1=OP.subtract)
                lnv = sb.tile([1, T2], F32, tag="lnstat", bufs=4)
                nc.scalar.activation(lnv[:], u[:], AF.Ln, scale=1.0 / D)
                rstd = sb.tile([1, T2], F32, tag="lnstat", bufs=4)
                nc.scalar.activation(rstd[:], lnv[:], AF.Exp, scale=-0.5)
                rstd_r = sb.tile([1, T2], F32R, tag="lnstat", bufs=4)
                nc.vector.tensor_copy(out=rstd_r[:], in_=rstd[:])
                mr_r = sb.tile([1, T2], F32R, tag="lnstat", bufs=4)
                nc.vector.scalar_tensor_tensor(
                    out=mr_r[:], in0=s1[:], scalar=1.0 / D, in1=rstd[:],
                    op0=OP.mult, op1=OP.mult)
                a = [sb.tile([128, T2], BF16, tag=f"{tag}{m}", bufs=1, name=f"a_{tag}_{next(_ln_ctr)}_{m}") for m in range(ND)]
                rb = ps.tile([128, T2], F32, tag="mm", bufs=2)
                nc.tensor.matmul(rb[:], onesM_r[:], rstd_r[:], start=True, stop=True)
                mb = ps.tile([128, T2], F32, tag="mm", bufs=2)
                nc.tensor.matmul(mb[:], onesM_r[:], mr_r[:], start=True, stop=True)
                rb_s = sb.tile([128, T2], F32, tag="rb_s", bufs=1)
                nc.vector.tensor_copy(out=rb_s[:], in_=rb[:])
                mb_s = sb.tile([128, T2], F32, tag="mb_s", bufs=1)
                nc.vector.tensor_copy(out=mb_s[:], in_=mb[:])
                for m in range(ND):
                    t1 = sb.tile([128, T2], F32, tag="lnt", bufs=2)
                    nc.vector.scalar_tensor_tensor(
                        out=t1[:], in0=src[m][:], scalar=1.0, in1=rb_s[:],
                        op0=OP.mult, op1=OP.mult)
                    nc.vector.scalar_tensor_tensor(
                        out=t1[:], in0=t1[:], scalar=1.0, in1=mb_s[:],
                        op0=OP.mult, op1=OP.subtract)
                    nc.scalar.activation(
                        a[m][:], t1[:], AF.Identity,
                        bias=b_ap[:, m : m + 1], scale=s_ap[:, m : m + 1])
                return a

            # ---------------- layers ----------------
            for l in range(nlayers):
                prm = sb.tile([128, NPRM], F32, tag="prm", bufs=2)
                nc.sync.dma_start(prm[:], prm_d[l])

                a1 = layer_norm(h, prm[:, P_LN1S : P_LN1S + ND], prm[:, P_LN1B : P_LN1B + ND])

                # V projection (token-major, 65-strided heads + ones col)
                kin_v = dram.tile([T2, 780], BF16, tag="kin_v", bufs=2)
                kout_v = dram.tile([2 * T2, 780], BF16, tag="kout_v", bufs=2)
                wv_sl = wp.tile([128, ND, D], BF16, tag="wv", bufs=1)
                nc.sync.dma_start(wv_sl[:], wv_d[l])
                for t in range(NT):
                    pv1 = ps.tile([128, T2], F32, tag="mm", bufs=2)
                    pv2 = ps.tile([128, 256], F32, tag="mm", bufs=2)
                    for k in range(ND):
                        lhs = a1[k][:, 128 * t : 128 * t + 128]
                        nc.tensor.matmul(pv1[:], lhs, wv_sl[:, k, 0:512], start=(k == 0), stop=(k == ND - 1))
                        nc.tensor.matmul(pv2[:], lhs, wv_sl[:, k, 512:768], start=(k == 0), stop=(k == ND - 1))
                    vc = sb.tile([128, 780], BF16, tag="vc", bufs=2)
                    vch = vc[:].rearrange("p (h e) -> p h e", e=65)
                    nc.vector.tensor_copy(
                        out=vch[:, 0:8, 0:64],
                        in_=pv1[:].rearrange("p (h e) -> p h e", e=64))
                    nc.vector.tensor_copy(
                        out=vch[:, 8:12, 0:64],
                        in_=pv2[:].rearrange("p (h e) -> p h e", e=64))
                    nc.vector.memset(vch[:, :, 64:65], 1.0)
                    nc.sync.dma_start(kin_v[128 * t : 128 * t + 128, :], vc[:])
                nc.gpsimd.collective_compute(
                    "AllGather", OP.bypass,
                    ins=[kin_v[:].opt()], outs=[kout_v[:].opt()], replica_groups=rg)

                # K projection (feature-major) -> two half AGs (heads 0-5 / 6-11)
                kin_k = [dram.tile([D // 2, T2], BF16, tag=f"kin_k{g}", bufs=2, name=f"kin_k{l}_{g}") for g in range(2)]
                kout_k = [dram.tile([D, T2], BF16, tag=f"kout_k{g}", bufs=2, name=f"kout_k{l}_{g}") for g in range(2)]
                for m in range(ND):
                    g, mg = m // 3, m % 3
                    wk_sl = wp.tile([128, ND, 128], BF16, tag="wk", bufs=2)
                    nc.sync.dma_start(wk_sl[:], wk_d[l, m])
                    pk = ps.tile([128, T2], F32, tag="mm", bufs=2)
                    for k in range(ND):
                        nc.tensor.matmul(pk[:], wk_sl[:, k], a1[k][:], start=(k == 0), stop=(k == ND - 1))
                    kc = sb.tile([128, T2], BF16, tag="kc", bufs=2)
                    nc.vector.tensor_copy(out=kc[:], in_=pk[:])
                    nc.sync.dma_start(
                        kin_k[g][:].rearrange("(ko ki) t -> ki ko t", ki=128)[:, mg], kc[:])
                    if mg == 2:
                        nc.gpsimd.collective_compute(
                            "AllGather", OP.bypass,
                            ins=[kin_k[g][:].opt()], outs=[kout_k[g][:].opt()],
                            replica_groups=rg)

                # Q projection (feature-major, stays local)
                q = [sb.tile([128, T2], BF16, tag=f"q{m}", bufs=1, name=f"q{l}_{m}") for m in range(ND)]
                for m in range(ND):
                    wq_sl = wp.tile([128, ND, 128], BF16, tag="wq", bufs=2)
                    nc.sync.dma_start(wq_sl[:], wq_d[l, m])
                    pq = ps.tile([128, T2], F32, tag="mm", bufs=2)
                    for k in range(ND):
                        nc.tensor.matmul(pq[:], wq_sl[:, k], a1[k][:], start=(k == 0), stop=(k == ND - 1))
                    nc.vector.tensor_copy(out=q[m][:], in_=pq[:])

                # gathered K (feature-major) / V-hat (token-major)
                # kg chunk layout: j = ND*half + ko
                kg = sb.tile([128, 2 * ND, T2], BF16, tag="kg", bufs=1)
                for g in range(2):
                    src_g = kout_k[g][:].rearrange("(hf ko ki) t -> ki hf ko t", ki=128, ko=3)
                    nc.sync.dma_start(kg[:, 3 * g : 3 * g + 3], src_g[:, 0])
                    nc.sync.dma_start(kg[:, ND + 3 * g : ND + 3 * g + 3], src_g[:, 1])
                vg = sb.tile([128, NK, 780], BF16, tag="vg", bufs=1)
                nc.sync.dma_start(vg[:], kout_v[:].rearrange("(to ti) f -> ti to f", ti=128))

                # attention: head pairs (2*hp, 2*hp+1) share feature chunk hp;
                # two denominator groups (pairs 0-2 / 3-5) for overlap
                o = [sb.tile([128, T2], BF16, tag=f"o{m}", bufs=1, name=f"o{l}_{m}") for m in range(ND)]
                dng = [sb.tile([6, T2], F32, tag=f"dn{g}", bufs=1, name=f"dn{l}_{g}") for g in range(2)]
                for hp in range(ND):
                    po2 = [ps.tile([65, T2], F32, tag="po", bufs=2, name=f"po_{l}_{hp}_{j}") for j in range(2)]
                    for c in range(NK):
                        # alternating 128-block split: global key block c lives on
                        # pair-member c%2 at its local block c//2; queries below
                        # local block c//2 never attend to it on either core
                        qlo = 128 * (c // 2)
                        mem, loc = c % 2, c // 2
                        s2j = ps.tile([128, 2, T2], F32, tag="s", bufs=2)
                        for j in range(2):
                            nc.tensor.matmul(
                                s2j[:, j, qlo:],
                                kg[64 * j : 64 * j + 64, ND * mem + hp, 128 * loc : 128 * loc + 128],
                                q[hp][64 * j : 64 * j + 64, qlo:],
                                start=True, stop=True)
                        p_t = sb.tile([128, 2, T2], BF16, tag="p", bufs=5)
                        nc.scalar.activation(p_t[:, :, qlo:], s2j[:, :, qlo:], AF.Exp, scale=SCALE)
                        for j in range(2):
                            # only the diagonal 128-col window ever needs masking
                            nc.vector.tensor_mul(
                                out=p_t[:, j, qlo : qlo + 128],
                                in0=p_t[:, j, qlo : qlo + 128], in1=msk[:, c, :])
                        to = 4 * mem + loc
                        for j in range(2):
                            hi = 2 * hp + j
                            nc.tensor.matmul(
                                po2[j][:, qlo:], vg[:, to, 65 * hi : 65 * hi + 65], p_t[:, j, qlo:],
                                start=(c == 0), stop=(c == NK - 1))
                    g = hp // 3
                    for j in range(2):
                        hi = 2 * hp + j
                        nc.vector.tensor_copy(out=o[hp][64 * j : 64 * j + 64, :], in_=po2[j][0:64, :])
                        dtmp = sb.tile([1, T2], F32, tag="dtmp", bufs=2)
                        nc.vector.tensor_copy(out=dtmp[:], in_=po2[j][64:65, :])
                        nc.sync.dma_start(dng[g][(hi - 6 * g) : (hi - 6 * g) + 1, :], dtmp[:])

                # normalize: 1/denom = exp(-ln(denom)); pair-head broadcast via selector
                for g, sel in ((0, selA_r), (1, selB_r)):
                    nc.scalar.activation(dng[g][:], dng[g][:], AF.Ln)
                    rec_r = sb.tile([6, T2], F32R, tag=f"recr{g}", bufs=1, name=f"recr{l}_{g}")
                    nc.scalar.activation(rec_r[:], dng[g][:], AF.Exp, scale=-1.0)
                    for m in range(3 * g, 3 * g + 3):
                        dnb = ps.tile([128, T2], F32, tag="mm", bufs=2)
                        nc.tensor.matmul(dnb[:], sel[:, m, :], rec_r[:], start=True, stop=True)
                        nc.vector.scalar_tensor_tensor(
                            out=o[m][:], in0=o[m][:], scalar=1.0,
                            in1=dnb[:], op0=OP.mult, op1=OP.mult)

                # output projection + residual
                for m in range(ND):
                    wo_sl = wp.tile([128, ND, 128], BF16, tag="wo", bufs=2)
                    nc.sync.dma_start(wo_sl[:], wo_d[l, m])
                    pw = ps.tile([128, T2], F32, tag="mm", bufs=2)
                    for k in range(ND):
                        nc.tensor.matmul(pw[:], wo_sl[:, k], o[k][:], start=(k == 0), stop=(k == ND - 1))
                    tt = sb.tile([128, T2], F32, tag="res", bufs=2)
                    nc.scalar.activation(tt[:], pw[:], AF.Identity, bias=prm[:, P_BO + m : P_BO + m + 1])
                    nc.vector.tensor_tensor(out=h[m][:], in0=h[m][:], in1=tt[:], op=OP.add)

                # FFN
                a2 = layer_norm(h, prm[:, P_LN2S : P_LN2S + ND], prm[:, P_LN2B : P_LN2B + ND])
                f = [sb.tile([128, T2], BF16, tag=f"f{fc}", bufs=1, name=f"f{l}_{fc}") for fc in range(NF)]
                for fc in range(NF):
                    w1_sl = wp.tile([128, ND, 128], BF16, tag="w1", bufs=3)
                    nc.sync.dma_start(w1_sl[:], w1_d[l, fc])
                    pf = ps.tile([128, T2], F32, tag="mm", bufs=2)
                    for k in range(ND):
                        nc.tensor.matmul(pf[:], w1_sl[:, k], a2[k][:], start=(k == 0), stop=(k == ND - 1))
                    nc.scalar.activation(f[fc][:], pf[:], AF.Relu, bias=prm[:, P_B1 + fc : P_B1 + fc + 1])
                for m in range(ND):
                    w2_sl = wp.tile([128, NF, 128], BF16, tag="w2", bufs=3)
                    nc.sync.dma_start(w2_sl[:], w2_d[l, m])
                    pg = ps.tile([128, T2], F32, tag="mm", bufs=2)
                    for k in range(NF):
                        nc.tensor.matmul(pg[:], w2_sl[:, k], f[k][:], start=(k == 0), stop=(k == NF - 1))
                    tt = sb.tile([128, T2], F32, tag="res", bufs=2)
                    nc.scalar.activation(tt[:], pg[:], AF.Identity, bias=prm[:, P_B2 + m : P_B2 + m + 1])
                    nc.vector.tensor_tensor(out=h[m][:], in0=h[m][:], in1=tt[:], op=OP.add)

            # ---------------- final LN + head ----------------
            hf_t = layer_norm(h, lnf_sb[:, 0:ND], lnf_sb[:, ND : 2 * ND], tag="a")
            for v in range(NV):
                wh_sl = wp.tile([128, ND, 512], BF16, tag="wh", bufs=2)
                nc.sync.dma_start(wh_sl[:], wh_d[v])
                o_dst = out_d[:, 512 * v : 512 * v + 512].rearrange("(to ti) f -> ti to f", ti=128)
                lg = sb.tile([128, NT, 512], BF16, tag="lg", bufs=2)
                for t in range(NT):
                    pl = ps.tile([128, 512], F32, tag="mm", bufs=2)
                    for k in range(ND):
                        nc.tensor.matmul(
                            pl[:], hf_t[k][:, 128 * t : 128 * t + 128], wh_sl[:, k],
                            start=(k == 0), stop=(k == ND - 1))
                    nc.vector.tensor_copy(out=lg[:, t, :], in_=pl[:])
                nc.sync.dma_start(o_dst[:], lg[:])

    import concourse.bacc as _bacc_mod
    _orig_gat = _bacc_mod.get_activation_tables
    _bacc_mod.get_activation_tables = _act_single_set_tables(_orig_gat)
    try:
        nc.compile()
    finally:
        _bacc_mod.get_activation_tables = _orig_gat
    if not nc.is_finalized():
        nc.finalize()
    return nc


def _prep_shared(inputs, nlayers):
    bf = ml_dtypes.bfloat16
    wq, wk, wv, wo = (np.asarray(inputs[k], np.float32) for k in ("wq", "wk", "wv", "wo"))
    w1, w2 = np.asarray(inputs["w1"], np.float32), np.asarray(inputs["w2"], np.float32)
    w_head = np.asarray(inputs["w_head"], np.float32)

    def lhst(w, nm, nk):
        # [L, nk*128, nm*128] -> [L, nm, 128, nk, 128] with [l,m,ki,ko,j] = w[l,128ko+ki,128m+j]
        return np.ascontiguousarray(
            w[:nlayers].reshape(nlayers, nk, 128, nm, 128).transpose(0, 3, 2, 1, 4)).astype(bf)

    d = {}
    d["wq_t"] = lhst(wq, ND, ND)
    d["wk_t"] = lhst(wk, ND, ND)
    d["wo_t"] = lhst(wo, ND, ND)
    d["w1_t"] = lhst(w1, NF, ND)
    d["w2_t"] = lhst(w2, ND, NF)
    d["wv_t"] = np.ascontiguousarray(
        wv[:nlayers].reshape(nlayers, ND, 128, D).transpose(0, 2, 1, 3)).astype(bf)
    whp = np.concatenate([w_head, np.zeros((D, VP - V), np.float32)], axis=1)
    d["wh_t"] = np.ascontiguousarray(
        whp.reshape(ND, 128, NV, 512).transpose(2, 1, 0, 3)).astype(bf)

    prm = np.zeros((nlayers, 128, NPRM), np.float32)

    def chunked(a):  # [L, 768] -> [L, 128, 6]
        return np.asarray(a, np.float32)[:nlayers].reshape(nlayers, -1, 128).transpose(0, 2, 1)

    prm[:, :, P_LN1S : P_LN1S + ND] = chunked(inputs["ln1_s"])
    prm[:, :, P_LN1B : P_LN1B + ND] = chunked(inputs["ln1_b"])
    prm[:, :, P_LN2S : P_LN2S + ND] = chunked(inputs["ln2_s"])
    prm[:, :, P_LN2B : P_LN2B + ND] = chunked(inputs["ln2_b"])
    prm[:, :, P_BO : P_BO + ND] = chunked(inputs["bo"])
    prm[:, :, P_B2 : P_B2 + ND] = chunked(inputs["b2"])
    prm[:, :, P_B1 : P_B1 + NF] = chunked(inputs["b1"])
    d["prm"] = np.ascontiguousarray(prm)

    lnf = np.zeros((128, 2 * ND), np.float32)
    lnf[:, 0:ND] = np.asarray(inputs["lnf_s"], np.float32).reshape(ND, 128).T
    lnf[:, ND : 2 * ND] = np.asarray(inputs["lnf_b"], np.float32).reshape(ND, 128).T
    d["lnf"] = np.ascontiguousarray(lnf)

    selA = np.zeros((6, ND, 128), np.float32)
    selB = np.zeros((6, ND, 128), np.float32)
    for hi in range(H):
        tgt = selA if hi < 6 else selB
        tgt[hi % 6, hi // 2, 64 * (hi % 2) : 64 * (hi % 2) + 64] = 1.0
    d["selA"] = selA
    d["selB"] = selB
    return d


_LAST_RESULTS = None


def kernel(x, tok_emb, pos_emb, wq, wk, wv, wo, bo, ln1_s, ln1_b,
           ln2_s, ln2_b, w1, b1, w2, b2, lnf_s, lnf_b, w_head, b_head,
           nlayers=L):
    global _LAST_RESULTS
    if nlayers not in _BUILT:
        _BUILT[nlayers] = _build(nlayers)
    nc = _BUILT[nlayers]

    inputs = dict(x=x, tok_emb=tok_emb, pos_emb=pos_emb, wq=wq, wk=wk, wv=wv,
                  wo=wo, bo=bo, ln1_s=ln1_s, ln1_b=ln1_b, ln2_s=ln2_s,
                  ln2_b=ln2_b, w1=w1, b1=b1, w2=w2, b2=b2, lnf_s=lnf_s,
                  lnf_b=lnf_b, w_head=w_head, b_head=b_head)
    shared = _prep_shared(inputs, nlayers)

    xi = np.asarray(x).astype(np.int64)
    te = np.asarray(tok_emb, np.float32)
    pe = np.asarray(pos_emb, np.float32)[:T]
    h0 = te[xi] + pe[None, :, :]  # [B, T, D] fp32

    in_maps = []
    for c in range(8):
        b, s = c // 2, c % 2
        # alternating 128-token-block split: core parity s owns global blocks
        # {s, s+2, s+4, s+6}; local block i <-> global block 2i+s
        tok_idx = np.arange(T).reshape(NK, 128)[s::2].reshape(T2)
        hc = np.ascontiguousarray(
            h0[b, tok_idx].T.reshape(ND, 128, T2).transpose(1, 0, 2))
        kk = np.arange(T).reshape(NK, 128)[:, :, None]  # global key pos [c, ki, 1]
        # diagonal window: key chunk c vs this core's local col block c//2
        # (global q block 2*(c//2)+s)
        qq = (128 * (2 * (np.arange(NK) // 2) + s))[:, None, None] + np.arange(128)[None, None, :]
        mc = (kk <= qq).astype(ml_dtypes.bfloat16)
        m = {"h0": hc, "masks": np.ascontiguousarray(mc)}
        m.update(shared)
        in_maps.append(m)

    res = bass_utils.run_bass_kernel_spmd(nc, in_maps, core_ids=list(range(8)))
    _LAST_RESULTS = res

    out = np.empty((B, T, V), np.float32)
    for c in range(8):
        b, s = c // 2, c % 2
        tok_idx = np.arange(T).reshape(NK, 128)[s::2].reshape(T2)
        out[b, tok_idx] = res.results[c]["logits"][:, :V].astype(np.float32)
    bh = np.asarray(b_head, np.float32)
    if np.any(bh):
        out += bh
    return out


if __name__ == "__main__":
    nl = int(os.environ.get("KERNEL_LAYERS", L))
    _build(nl)
    print("build ok", nl)



# revision 28
# speedup vs baseline: 1.1375x; 1.1375x over previous
"""Bass/Trainium2 kernel for a 6-layer dense transformer LM (BigramLanguageModel).

Sharding (8 cores): core c = (batch b = c//2, seq-half s = c%2).
Each core owns 512 tokens of one batch: runs the full 6-layer transformer on its
tokens, exchanging per-layer K/V with its pair core via pairwise AllGathers
(replica groups [[0,1],[2,3],[4,5],[6,7]]), then computes logits for its tokens
over the FULL vocab. Output is assembled on the host.

Device-side layout choices:
  - Activations are feature-major [D(6x128 partition chunks), T2=512(free)], so
    every projection uses the natural weight layout as matmul lhsT and produces
    feature-major output with zero transposes anywhere.
  - Attention computes S^T[k,q] = K-slices^T @ Q so softmax runs along the free
    dim; V is computed token-major so it is directly the PV lhsT; a built-in
    ones-column in V yields the softmax denominator in the same matmul.
  - Softmax skips max-subtraction (|scores*scale| < ~3 for this model family);
    1/x and 1/sqrt(x) are computed as exp(-ln(x)) / exp(-0.5 ln(x)) on the ACT
    engine (fast, and keeps a single activation-table set resident).
  - Matmuls in bf16; residual stream fp32; LN statistics via fp32r PE matmuls.
"""

import os
import sys

for _p in ("/opt/trn_rl_repo", "/root/.axon_site/_ro/trn_rl_repo"):
    if os.path.isdir(_p) and _p not in sys.path:
        sys.path.insert(0, _p)

import numpy as np
import ml_dtypes

import concourse.bass as bass
import concourse.mybir as mybir
import concourse.tile as tile
from concourse import bacc
from concourse import bass_utils

F32 = mybir.dt.float32
F32R = mybir.dt.float32r
BF16 = mybir.dt.bfloat16
AF = mybir.ActivationFunctionType
OP = mybir.AluOpType

L = 6
D = 768
H = 12
HD = 64
FF = 3072
V = 32000
VP = 32256  # padded vocab: 63 slices of 512
T = 1024
T2 = 512
B = 4
ND = D // 128   # 6 feature chunks
NF = FF // 128  # 24 ff chunks
NT = T2 // 128  # 4 own-token chunks
NK = T // 128   # 8 global key chunks
NV = VP // 512  # 63 vocab slices
SCALE = HD ** -0.5
EPS = 1e-5

# param columns in the packed per-layer param tile [128, 60]
P_LN1S, P_LN1B, P_LN2S, P_LN2B, P_BO, P_B2, P_B1 = 0, 6, 12, 18, 24, 30, 36
NPRM = 60

_BUILT = {}


def _act_single_set_tables(orig_get_tables):
    """All ACT funcs used here (Ln, Exp, Relu, Identity, Copy) live together in
    the `natural_log_exp_and_others` set; the default per-func set choice picks
    the first matching set and thrashes ACT_TABLE_LOADs (50 loads, ~64us, all
    serial on ACT). Strip those funcs from every other set so the load-insertion
    fixpoint has exactly one choice. Set ids (dict order) are preserved."""
    def patched(arch):
        t = dict(orig_get_tables(arch))
        keep = t["natural_log_exp_and_others"]
        return {
            name: (funcs if name == "natural_log_exp_and_others" else funcs - keep)
            for name, funcs in t.items()
        }
    return patched


def _build(nlayers=L):
    nc = bacc.Bacc("TRN2", target_bir_lowering=False, debug=False)

    h0_d = nc.dram_tensor("h0", [128, ND, T2], F32, kind="ExternalInput")
    msk_d = nc.dram_tensor("masks", [NK, 128, 128], BF16, kind="ExternalInput")
    wq_d = nc.dram_tensor("wq_t", [nlayers, ND, 128, ND, 128], BF16, kind="ExternalInput")
    wk_d = nc.dram_tensor("wk_t", [nlayers, ND, 128, ND, 128], BF16, kind="ExternalInput")
    wo_d = nc.dram_tensor("wo_t", [nlayers, ND, 128, ND, 128], BF16, kind="ExternalInput")
    wv_d = nc.dram_tensor("wv_t", [nlayers, 128, ND, D], BF16, kind="ExternalInput")
    w1_d = nc.dram_tensor("w1_t", [nlayers, NF, 128, ND, 128], BF16, kind="ExternalInput")
    w2_d = nc.dram_tensor("w2_t", [nlayers, ND, 128, NF, 128], BF16, kind="ExternalInput")
    wh_d = nc.dram_tensor("wh_t", [NV, 128, ND, 512], BF16, kind="ExternalInput")
    prm_d = nc.dram_tensor("prm", [nlayers, 128, NPRM], F32, kind="ExternalInput")
    lnf_d = nc.dram_tensor("lnf", [128, 2 * ND], F32, kind="ExternalInput")
    selA_d = nc.dram_tensor("selA", [6, ND, 128], F32, kind="ExternalInput")
    selB_d = nc.dram_tensor("selB", [6, ND, 128], F32, kind="ExternalInput")
    out_d = nc.dram_tensor("logits", [T2, VP], BF16, kind="ExternalOutput")

    rg = [[0, 1], [2, 3], [4, 5], [6, 7]]

    with tile.TileContext(nc) as tc:
        with tc.tile_pool(name="pers", bufs=1) as pers, \
             tc.tile_pool(name="sb", bufs=1) as sb, \
             tc.tile_pool(name="w", bufs=1) as wp, \
             tc.tile_pool(name="ps", bufs=1, space="PSUM") as ps, \
             tc.tile_pool(name="dram", bufs=1, space="DRAM") as dram:

            # ---------------- persistent tiles ----------------
            h = [pers.tile([128, T2], F32, name=f"h{m}") for m in range(ND)]
            for m in range(ND):
                nc.sync.dma_start(h[m][:], h0_d[:, m, :])
            msk = pers.tile([128, NK, 128], BF16)
            nc.sync.dma_start(msk[:], msk_d[:].rearrange("c p t -> p c t"))

            ones_f = pers.tile([128, 1], F32)
            nc.vector.memset(ones_f[:], 1.0)
            ones_r = pers.tile([128, 1], F32R)          # LN sum lhsT [K=128, M=1]
            nc.vector.tensor_copy(out=ones_r[:], in_=ones_f[:])
            onesM_f = pers.tile([1, 128], F32)
            nc.vector.memset(onesM_f[:], 1.0)
            onesM_r = pers.tile([1, 128], F32R)         # bcast lhsT [K=1, M<=128]
            nc.vector.tensor_copy(out=onesM_r[:], in_=onesM_f[:])

            lnf_sb = pers.tile([128, 2 * ND], F32)
            nc.sync.dma_start(lnf_sb[:], lnf_d[:])
            selA_r = pers.tile([6, ND, 128], F32R)
            selB_r = pers.tile([6, ND, 128], F32R)
            for _sd, _sr in ((selA_d, selA_r), (selB_d, selB_r)):
                sel_f = sb.tile([6, ND, 128], F32, tag="self", bufs=1, name=f"self_{_sd.name}")
                nc.sync.dma_start(sel_f[:], _sd[:])
                nc.vector.tensor_copy(out=_sr[:], in_=sel_f[:])

            import itertools
            _ln_ctr = itertools.count()

            # ---------------- helpers ----------------
            def layer_norm(src, s_ap, b_ap, tag="a"):
                """src: list of ND [128, T2] fp32 tiles -> list of ND bf16 tiles.
                s_ap/b_ap: [128, ND] fp32 scale/bias tiles.
                rstd = exp(-0.5*ln(var+eps)) keeps the serial chain short."""
                # stats borrow the attention po-tag banks (never live at the
                # same time: LN stats need the full residual, which needs all
                # attention outputs)
                s1 = ps.tile([1, T2], F32, tag="po", bufs=2)
                s2 = ps.tile([1, T2], F32, tag="po", bufs=2)
                for m in range(ND):
                    h_r = sb.tile([128, T2], F32R, tag="h_r", bufs=2)
                    nc.vector.tensor_copy(out=h_r[:], in_=src[m][:])
                    hsq = sb.tile([128, T2], F32R, tag="hsq", bufs=2)
                    nc.vector.tensor_mul(out=hsq[:], in0=src[m][:], in1=src[m][:])
                    nc.tensor.matmul(s1[:], ones_r[:], h_r[:], start=(m == 0), stop=(m == ND - 1))
                    nc.tensor.matmul(s2[:], ones_r[:], hsq[:], start=(m == 0), stop=(m == ND - 1))
                # u = (s2 + D*eps) - s1^2/D;  var+eps = u/D (Ln's free affine
                # applies the 1/D scale; Square's applies the 1/D inside)
                ssq = sb.tile([1, T2], F32, tag="lnstat", bufs=6)
                nc.scalar.activation(ssq[:], s1[:], AF.Square, scale=float(D) ** -0.5)
                u = sb.tile([1, T2], F32, tag="lnstat", bufs=6)
                nc.vector.scalar_tensor_tensor(
                    out=u[:], in0=s2[:], scalar=float(D * EPS), in1=ssq[:],
                    op0=OP.add, op1=OP.subtract)
                lnv = sb.tile([1, T2], F32, tag="lnstat", bufs=6)
                nc.scalar.activation(lnv[:], u[:], AF.Ln, scale=1.0 / D)
                rstd = sb.tile([1, T2], F32, tag="lnstat", bufs=6)
                nc.scalar.activation(rstd[:], lnv[:], AF.Exp, scale=-0.5)
                rstd_r = sb.tile([1, T2], F32R, tag="lnstat", bufs=6)
                nc.vector.tensor_copy(out=rstd_r[:], in_=rstd[:])
                mr_r = sb.tile([1, T2], F32R, tag="lnstat", bufs=6)
                nc.vector.scalar_tensor_tensor(
                    out=mr_r[:], in0=s1[:], scalar=1.0 / D, in1=rstd[:],
                    op0=OP.mult, op1=OP.mult)
                a = [sb.tile([128, T2], BF16, tag=f"{tag}{m}", bufs=1, name=f"a_{tag}_{next(_ln_ctr)}_{m}") for m in range(ND)]
                rb = ps.tile([128, T2], F32, tag="mm", bufs=2)
                nc.tensor.matmul(rb[:], onesM_r[:], rstd_r[:], start=True, stop=True)
                mb = ps.tile([128, T2], F32, tag="mm", bufs=2)
                nc.tensor.matmul(mb[:], onesM_r[:], mr_r[:], start=True, stop=True)
                rb_s = sb.tile([128, T2], F32, tag="rb_s", bufs=1)
                nc.vector.tensor_copy(out=rb_s[:], in_=rb[:])
                mb_s = sb.tile([128, T2], F32, tag="mb_s", bufs=1)
                nc.vector.tensor_copy(out=mb_s[:], in_=mb[:])
                for m in range(ND):
                    t1 = sb.tile([128, T2], F32, tag="lnt", bufs=2)
                    nc.vector.scalar_tensor_tensor(
                        out=t1[:], in0=src[m][:], scalar=1.0, in1=rb_s[:],
                        op0=OP.mult, op1=OP.mult)
                    nc.vector.scalar_tensor_tensor(
                        out=t1[:], in0=t1[:], scalar=1.0, in1=mb_s[:],
                        op0=OP.mult, op1=OP.subtract)
                    nc.scalar.activation(
                        a[m][:], t1[:], AF.Identity,
                        bias=b_ap[:, m : m + 1], scale=s_ap[:, m : m + 1])
                return a

            # ---------------- layers ----------------
            for l in range(nlayers):
                prm = sb.tile([128, NPRM], F32, tag="prm", bufs=2)
                nc.sync.dma_start(prm[:], prm_d[l])

                a1 = layer_norm(h, prm[:, P_LN1S : P_LN1S + ND], prm[:, P_LN1B : P_LN1B + ND])

                # V projection (token-major, 65-strided heads + ones col);
                # AllGather in two token-halves so PV for early key chunks
                # unblocks before the whole V is exchanged
                kin_v = [dram.tile([256, 780], BF16, tag=f"kin_v{hh}", bufs=2,
                                   name=f"kin_v{l}_{hh}") for hh in range(2)]
                kout_v = [dram.tile([2, 256, 780], BF16, tag=f"kout_v{hh}", bufs=2,
                                    name=f"kout_v{l}_{hh}") for hh in range(2)]
                wv_sl = wp.tile([128, ND, D], BF16, tag="wv", bufs=1)
                nc.sync.dma_start(wv_sl[:], wv_d[l])
                for t in range(NT):
                    pv1 = ps.tile([128, T2], F32, tag="mm", bufs=2)
                    pv2 = ps.tile([128, 256], F32, tag="mm", bufs=2)
                    for k in range(ND):
                        lhs = a1[k][:, 128 * t : 128 * t + 128]
                        nc.tensor.matmul(pv1[:], lhs, wv_sl[:, k, 0:512], start=(k == 0), stop=(k == ND - 1))
                        nc.tensor.matmul(pv2[:], lhs, wv_sl[:, k, 512:768], start=(k == 0), stop=(k == ND - 1))
                    vc = sb.tile([128, 780], BF16, tag="vc", bufs=2)
                    vch = vc[:].rearrange("p (h e) -> p h e", e=65)
                    nc.vector.tensor_copy(
                        out=vch[:, 0:8, 0:64],
                        in_=pv1[:].rearrange("p (h e) -> p h e", e=64))
                    nc.vector.tensor_copy(
                        out=vch[:, 8:12, 0:64],
                        in_=pv2[:].rearrange("p (h e) -> p h e", e=64))
                    nc.vector.memset(vch[:, :, 64:65], 1.0)
                    nc.sync.dma_start(kin_v[t // 2][128 * (t % 2) : 128 * (t % 2) + 128, :], vc[:])
                    if t % 2 == 1:
                        nc.gpsimd.collective_compute(
                            "AllGather", OP.bypass,
                            ins=[kin_v[t // 2][:].opt()], outs=[kout_v[t // 2][:].opt()],
                            replica_groups=rg)

                # K projection (feature-major) -> two half AGs (heads 0-5 / 6-11)
                kin_k = [dram.tile([D // 2, T2], BF16, tag=f"kin_k{g}", bufs=2, name=f"kin_k{l}_{g}") for g in range(2)]
                kout_k = [dram.tile([D, T2], BF16, tag=f"kout_k{g}", bufs=2, name=f"kout_k{l}_{g}") for g in range(2)]
                for m in range(ND):
                    g, mg = m // 3, m % 3
                    wk_sl = wp.tile([128, ND, 128], BF16, tag="wk", bufs=2)
                    nc.sync.dma_start(wk_sl[:], wk_d[l, m])
                    pk = ps.tile([128, T2], F32, tag="mm", bufs=2)
                    for k in range(ND):
                        nc.tensor.matmul(pk[:], wk_sl[:, k], a1[k][:], start=(k == 0), stop=(k == ND - 1))
                    kc = sb.tile([128, T2], BF16, tag="kc", bufs=2)
                    nc.vector.tensor_copy(out=kc[:], in_=pk[:])
                    nc.sync.dma_start(
                        kin_k[g][:].rearrange("(ko ki) t -> ki ko t", ki=128)[:, mg], kc[:])
                    if mg == 2:
                        nc.gpsimd.collective_compute(
                            "AllGather", OP.bypass,
                            ins=[kin_k[g][:].opt()], outs=[kout_k[g][:].opt()],
                            replica_groups=rg)

                # Q projection (feature-major, stays local)
                q = [sb.tile([128, T2], BF16, tag=f"q{m}", bufs=1, name=f"q{l}_{m}") for m in range(ND)]
                for m in range(ND):
                    wq_sl = wp.tile([128, ND, 128], BF16, tag="wq", bufs=2)
                    nc.sync.dma_start(wq_sl[:], wq_d[l, m])
                    pq = ps.tile([128, T2], F32, tag="mm", bufs=2)
                    for k in range(ND):
                        nc.tensor.matmul(pq[:], wq_sl[:, k], a1[k][:], start=(k == 0), stop=(k == ND - 1))
                    nc.vector.tensor_copy(out=q[m][:], in_=pq[:])

                # gathered K (feature-major) / V-hat (token-major)
                # kg chunk layout: j = ND*half + ko
                kg = sb.tile([128, 2 * ND, T2], BF16, tag="kg", bufs=1)
                for g in range(2):
                    src_g = kout_k[g][:].rearrange("(hf ko ki) t -> ki hf ko t", ki=128, ko=3)
                    nc.sync.dma_start(kg[:, 3 * g : 3 * g + 3], src_g[:, 0])
                    nc.sync.dma_start(kg[:, ND + 3 * g : ND + 3 * g + 3], src_g[:, 1])
                vg = sb.tile([128, NK, 780], BF16, tag="vg", bufs=1)
                for hh in range(2):
                    src_v = kout_v[hh][:].rearrange("g (lo ti) f -> ti g lo f", ti=128)
                    for mem in range(2):
                        nc.sync.dma_start(
                            vg[:, 4 * mem + 2 * hh : 4 * mem + 2 * hh + 2], src_v[:, mem])

                # attention: head pairs (2*hp, 2*hp+1) share feature chunk hp;
                # two denominator groups (pairs 0-2 / 3-5) for overlap
                o = [sb.tile([128, T2], BF16, tag=f"o{m}", bufs=1, name=f"o{l}_{m}") for m in range(ND)]
                dng = [sb.tile([6, T2], F32, tag=f"dn{g}", bufs=1, name=f"dn{l}_{g}") for g in range(2)]
                for hp in range(ND):
                    po2 = [ps.tile([65, T2], F32, tag="po", bufs=2, name=f"po_{l}_{hp}_{j}") for j in range(2)]
                    for c in range(NK):
                        # alternating 128-block split: global key block c lives on
                        # pair-member c%2 at its local block c//2; queries below
                        # local block c//2 never attend to it on either core
                        qlo = 128 * (c // 2)
                        mem, loc = c % 2, c // 2
                        s2j = ps.tile([128, 2, T2], F32, tag="s", bufs=2)
                        for j in range(2):
                            nc.tensor.matmul(
                                s2j[:, j, qlo:],
                                kg[64 * j : 64 * j + 64, ND * mem + hp, 128 * loc : 128 * loc + 128],
                                q[hp][64 * j : 64 * j + 64, qlo:],
                                start=True, stop=True)
                        p_t = sb.tile([128, 2, T2], BF16, tag="p", bufs=5)
                        nc.scalar.activation(p_t[:, :, qlo:], s2j[:, :, qlo:], AF.Exp, scale=SCALE)
                        for j in range(2):
                            # only the diagonal 128-col window ever needs masking
                            nc.vector.tensor_mul(
                                out=p_t[:, j, qlo : qlo + 128],
                                in0=p_t[:, j, qlo : qlo + 128], in1=msk[:, c, :])
                        to = 4 * mem + loc
                        for j in range(2):
                            hi = 2 * hp + j
                            nc.tensor.matmul(
                                po2[j][:, qlo:], vg[:, to, 65 * hi : 65 * hi + 65], p_t[:, j, qlo:],
                                start=(c == 0), stop=(c == NK - 1))
                    g = hp // 3
                    for j in range(2):
                        hi = 2 * hp + j
                        nc.vector.tensor_copy(out=o[hp][64 * j : 64 * j + 64, :], in_=po2[j][0:64, :])
                        dtmp = sb.tile([1, T2], F32, tag="dtmp", bufs=2)
                        nc.vector.tensor_copy(out=dtmp[:], in_=po2[j][64:65, :])
                        nc.sync.dma_start(dng[g][(hi - 6 * g) : (hi - 6 * g) + 1, :], dtmp[:])

                # normalize: 1/denom = exp(-ln(denom)); pair-head broadcast via selector
                for g, sel in ((0, selA_r), (1, selB_r)):
                    nc.scalar.activation(dng[g][:], dng[g][:], AF.Ln)
                    rec_r = sb.tile([6, T2], F32R, tag=f"recr{g}", bufs=1, name=f"recr{l}_{g}")
                    nc.scalar.activation(rec_r[:], dng[g][:], AF.Exp, scale=-1.0)
                    for m in range(3 * g, 3 * g + 3):
                        dnb = ps.tile([128, T2], F32, tag="mm", bufs=2)
                        nc.tensor.matmul(dnb[:], sel[:, m, :], rec_r[:], start=True, stop=True)
                        nc.vector.scalar_tensor_tensor(
                            out=o[m][:], in0=o[m][:], scalar=1.0,
                            in1=dnb[:], op0=OP.mult, op1=OP.mult)

                # output projection + residual
                for m in range(ND):
                    wo_sl = wp.tile([128, ND, 128], BF16, tag="wo", bufs=2)
                    nc.sync.dma_start(wo_sl[:], wo_d[l, m])
                    pw = ps.tile([128, T2], F32, tag="mm", bufs=2)
                    for k in range(ND):
                        nc.tensor.matmul(pw[:], wo_sl[:, k], o[k][:], start=(k == 0), stop=(k == ND - 1))
                    tt = sb.tile([128, T2], F32, tag="res", bufs=2)
                    nc.scalar.activation(tt[:], pw[:], AF.Identity, bias=prm[:, P_BO + m : P_BO + m + 1])
                    nc.vector.tensor_tensor(out=h[m][:], in0=h[m][:], in1=tt[:], op=OP.add)

                # FFN
                a2 = layer_norm(h, prm[:, P_LN2S : P_LN2S + ND], prm[:, P_LN2B : P_LN2B + ND])
                f = [sb.tile([128, T2], BF16, tag=f"f{fc}", bufs=1, name=f"f{l}_{fc}") for fc in range(NF)]
                for fc in range(NF):
                    w1_sl = wp.tile([128, ND, 128], BF16, tag="w1", bufs=3)
                    nc.sync.dma_start(w1_sl[:], w1_d[l, fc])
                    pf = ps.tile([128, T2], F32, tag="mm", bufs=2)
                    for k in range(ND):
                        nc.tensor.matmul(pf[:], w1_sl[:, k], a2[k][:], start=(k == 0), stop=(k == ND - 1))
                    nc.scalar.activation(f[fc][:], pf[:], AF.Relu, bias=prm[:, P_B1 + fc : P_B1 + fc + 1])
                for m in range(ND):
                    w2_sl = wp.tile([128, NF, 128], BF16, tag="w2", bufs=3)
                    nc.sync.dma_start(w2_sl[:], w2_d[l, m])
                    pg = ps.tile([128, T2], F32, tag="mm", bufs=2)
                    for k in range(NF):
                        nc.tensor.matmul(pg[:], w2_sl[:, k], f[k][:], start=(k == 0), stop=(k == NF - 1))
                    tt = sb.tile([128, T2], F32, tag="res", bufs=2)
                    nc.scalar.activation(tt[:], pg[:], AF.Identity, bias=prm[:, P_B2 + m : P_B2 + m + 1])
                    nc.vector.tensor_tensor(out=h[m][:], in0=h[m][:], in1=tt[:], op=OP.add)

            # ---------------- final LN + head ----------------
            hf_t = layer_norm(h, lnf_sb[:, 0:ND], lnf_sb[:, ND : 2 * ND], tag="a")
            for v in range(NV):
                wh_sl = wp.tile([128, ND, 512], BF16, tag="wh", bufs=2)
                nc.sync.dma_start(wh_sl[:], wh_d[v])
                o_dst = out_d[:, 512 * v : 512 * v + 512].rearrange("(to ti) f -> ti to f", ti=128)
                for t in range(NT):
                    pl = ps.tile([128, 512], F32, tag="mm", bufs=2)
                    for k in range(ND):
                        nc.tensor.matmul(
                            pl[:], hf_t[k][:, 128 * t : 128 * t + 128], wh_sl[:, k],
                            start=(k == 0), stop=(k == ND - 1))
                    lg = sb.tile([128, 512], BF16, tag="lg", bufs=4)
                    nc.vector.tensor_copy(out=lg[:], in_=pl[:])
                    nc.sync.dma_start(o_dst[:, t], lg[:])

    import concourse.bacc as _bacc_mod
    _orig_gat = _bacc_mod.get_activation_tables
    _bacc_mod.get_activation_tables = _act_single_set_tables(_orig_gat)
    try:
        nc.compile()
    finally:
        _bacc_mod.get_activation_tables = _orig_gat
    if not nc.is_finalized():
        nc.finalize()
    return nc


def _prep_shared(inputs, nlayers):
    bf = ml_dtypes.bfloat16
    wq, wk, wv, wo = (np.asarray(inputs[k], np.float32) for k in ("wq", "wk", "wv", "wo"))
    w1, w2 = np.asarray(inputs["w1"], np.float32), np.asarray(inputs["w2"], np.float32)
    w_head = np.asarray(inputs["w_head"], np.float32)

    def lhst(w, nm, nk):
        # [L, nk*128, nm*128] -> [L, nm, 128, nk, 128] with [l,m,ki,ko,j] = w[l,128ko+ki,128m+j]
        return np.ascontiguousarray(
            w[:nlayers].reshape(nlayers, nk, 128, nm, 128).transpose(0, 3, 2, 1, 4)).astype(bf)

    d = {}
    d["wq_t"] = lhst(wq, ND, ND)
    d["wk_t"] = lhst(wk, ND, ND)
    d["wo_t"] = lhst(wo, ND, ND)
    d["w1_t"] = lhst(w1, NF, ND)
    d["w2_t"] = lhst(w2, ND, NF)
    d["wv_t"] = np.ascontiguousarray(
        wv[:nlayers].reshape(nlayers, ND, 128, D).transpose(0, 2, 1, 3)).astype(bf)
    whp = np.concatenate([w_head, np.zeros((D, VP - V), np.float32)], axis=1)
    d["wh_t"] = np.ascontiguousarray(
        whp.reshape(ND, 128, NV, 512).transpose(2, 1, 0, 3)).astype(bf)

    prm = np.zeros((nlayers, 128, NPRM), np.float32)

    def chunked(a):  # [L, 768] -> [L, 128, 6]
        return np.asarray(a, np.float32)[:nlayers].reshape(nlayers, -1, 128).transpose(0, 2, 1)

    prm[:, :, P_LN1S : P_LN1S + ND] = chunked(inputs["ln1_s"])
    prm[:, :, P_LN1B : P_LN1B + ND] = chunked(inputs["ln1_b"])
    prm[:, :, P_LN2S : P_LN2S + ND] = chunked(inputs["ln2_s"])
    prm[:, :, P_LN2B : P_LN2B + ND] = chunked(inputs["ln2_b"])
    prm[:, :, P_BO : P_BO + ND] = chunked(inputs["bo"])
    prm[:, :, P_B2 : P_B2 + ND] = chunked(inputs["b2"])
    prm[:, :, P_B1 : P_B1 + NF] = chunked(inputs["b1"])
    d["prm"] = np.ascontiguousarray(prm)

    lnf = np.zeros((128, 2 * ND), np.float32)
    lnf[:, 0:ND] = np.asarray(inputs["lnf_s"], np.float32).reshape(ND, 128).T
    lnf[:, ND : 2 * ND] = np.asarray(inputs["lnf_b"], np.float32).reshape(ND, 128).T
    d["lnf"] = np.ascontiguousarray(lnf)

    selA = np.zeros((6, ND, 128), np.float32)
    selB = np.zeros((6, ND, 128), np.float32)
    for hi in range(H):
        tgt = selA if hi < 6 else selB
        tgt[hi % 6, hi // 2, 64 * (hi % 2) : 64 * (hi % 2) + 64] = 1.0
    d["selA"] = selA
    d["selB"] = selB
    return d


_LAST_RESULTS = None


def kernel(x, tok_emb, pos_emb, wq, wk, wv, wo, bo, ln1_s, ln1_b,
           ln2_s, ln2_b, w1, b1, w2, b2, lnf_s, lnf_b, w_head, b_head,
           nlayers=L):
    global _LAST_RESULTS
    if nlayers not in _BUILT:
        _BUILT[nlayers] = _build(nlayers)
    nc = _BUILT[nlayers]

    inputs = dict(x=x, tok_emb=tok_emb, pos_emb=pos_emb, wq=wq, wk=wk, wv=wv,
                  wo=wo, bo=bo, ln1_s=ln1_s, ln1_b=ln1_b, ln2_s=ln2_s,
                  ln2_b=ln2_b, w1=w1, b1=b1, w2=w2, b2=b2, lnf_s=lnf_s,
                  lnf_b=lnf_b, w_head=w_head, b_head=b_head)
    shared = _prep_shared(inputs, nlayers)

    xi = np.asarray(x).astype(np.int64)
    te = np.asarray(tok_emb, np.float32)
    pe = np.asarray(pos_emb, np.float32)[:T]
    h0 = te[xi] + pe[None, :, :]  # [B, T, D] fp32

    in_maps = []
    for c in range(8):
        b, s = c // 2, c % 2
        # alternating 128-token-block split: core parity s owns global blocks
        # {s, s+2, s+4, s+6}; local block i <-> global block 2i+s
        tok_idx = np.arange(T).reshape(NK, 128)[s::2].reshape(T2)
        hc = np.ascontiguousarray(
            h0[b, tok_idx].T.reshape(ND, 128, T2).transpose(1, 0, 2))
        kk = np.arange(T).reshape(NK, 128)[:, :, None]  # global key pos [c, ki, 1]
        # diagonal window: key chunk c vs this core's local col block c//2
        # (global q block 2*(c//2)+s)
        qq = (128 * (2 * (np.arange(NK) // 2) + s))[:, None, None] + np.arange(128)[None, None, :]
        mc = (kk <= qq).astype(ml_dtypes.bfloat16)
        m = {"h0": hc, "masks": np.ascontiguousarray(mc)}
        m.update(shared)
        in_maps.append(m)

    res = bass_utils.run_bass_kernel_spmd(nc, in_maps, core_ids=list(range(8)))
    _LAST_RESULTS = res

    out = np.empty((B, T, V), np.float32)
    for c in range(8):
        b, s = c // 2, c % 2
        tok_idx = np.arange(T).reshape(NK, 128)[s::2].reshape(T2)
        out[b, tok_idx] = res.results[c]["logits"][:, :V].astype(np.float32)
    bh = np.asarray(b_head, np.float32)
    if np.any(bh):
        out += bh
    return out


if __name__ == "__main__":
    nl = int(os.environ.get("KERNEL_LAYERS", L))
    _build(nl)
    print("build ok", nl)



# revision 30
# speedup vs baseline: 1.1469x; 1.0082x over previous
"""Bass/Trainium2 kernel for a 6-layer dense transformer LM (BigramLanguageModel).

Sharding (8 cores): core c = (batch b = c//2, seq-half s = c%2).
Each core owns 512 tokens of one batch: runs the full 6-layer transformer on its
tokens, exchanging per-layer K/V with its pair core via pairwise AllGathers
(replica groups [[0,1],[2,3],[4,5],[6,7]]), then computes logits for its tokens
over the FULL vocab. Output is assembled on the host.

Device-side layout choices:
  - Activations are feature-major [D(6x128 partition chunks), T2=512(free)], so
    every projection uses the natural weight layout as matmul lhsT and produces
    feature-major output with zero transposes anywhere.
  - Attention computes S^T[k,q] = K-slices^T @ Q so softmax runs along the free
    dim; V is computed token-major so it is directly the PV lhsT; a built-in
    ones-column in V yields the softmax denominator in the same matmul.
  - Softmax skips max-subtraction (|scores*scale| < ~3 for this model family);
    1/x and 1/sqrt(x) are computed as exp(-ln(x)) / exp(-0.5 ln(x)) on the ACT
    engine (fast, and keeps a single activation-table set resident).
  - Matmuls in bf16; residual stream fp32; LN statistics via fp32r PE matmuls.
"""

import os
import sys

for _p in ("/opt/trn_rl_repo", "/root/.axon_site/_ro/trn_rl_repo"):
    if os.path.isdir(_p) and _p not in sys.path:
        sys.path.insert(0, _p)

import numpy as np
import ml_dtypes

import concourse.bass as bass
import concourse.mybir as mybir
import concourse.tile as tile
from concourse import bacc
from concourse import bass_utils

F32 = mybir.dt.float32
F32R = mybir.dt.float32r
BF16 = mybir.dt.bfloat16
AF = mybir.ActivationFunctionType
OP = mybir.AluOpType

L = 6
D = 768
H = 12
HD = 64
FF = 3072
V = 32000
VP = 32256  # padded vocab: 63 slices of 512
T = 1024
T2 = 512
B = 4
ND = D // 128   # 6 feature chunks
NF = FF // 128  # 24 ff chunks
NT = T2 // 128  # 4 own-token chunks
NK = T // 128   # 8 global key chunks
NV = VP // 512  # 63 vocab slices
SCALE = HD ** -0.5
EPS = 1e-5

# param columns in the packed per-layer param tile [128, 60]
P_LN1S, P_LN1B, P_LN2S, P_LN2B, P_BO, P_B2, P_B1 = 0, 6, 12, 18, 24, 30, 36
NPRM = 60

_BUILT = {}


def _act_single_set_tables(orig_get_tables):
    """All ACT funcs used here (Ln, Exp, Relu, Identity, Copy) live together in
    the `natural_log_exp_and_others` set; the default per-func set choice picks
    the first matching set and thrashes ACT_TABLE_LOADs (50 loads, ~64us, all
    serial on ACT). Strip those funcs from every other set so the load-insertion
    fixpoint has exactly one choice. Set ids (dict order) are preserved."""
    def patched(arch):
        t = dict(orig_get_tables(arch))
        keep = t["natural_log_exp_and_others"]
        return {
            name: (funcs if name == "natural_log_exp_and_others" else funcs - keep)
            for name, funcs in t.items()
        }
    return patched


def _build(nlayers=L):
    nc = bacc.Bacc("TRN2", target_bir_lowering=False, debug=False)

    h0_d = nc.dram_tensor("h0", [128, ND, T2], F32, kind="ExternalInput")
    msk_d = nc.dram_tensor("masks", [NK, 128, 128], BF16, kind="ExternalInput")
    wq_d = nc.dram_tensor("wq_t", [nlayers, ND, 128, ND, 128], BF16, kind="ExternalInput")
    wk_d = nc.dram_tensor("wk_t", [nlayers, ND, 128, ND, 128], BF16, kind="ExternalInput")
    wo_d = nc.dram_tensor("wo_t", [nlayers, ND, 128, ND, 128], BF16, kind="ExternalInput")
    wv_d = nc.dram_tensor("wv_t", [nlayers, 128, ND, D], BF16, kind="ExternalInput")
    w1_d = nc.dram_tensor("w1_t", [nlayers, NF, 128, ND, 128], BF16, kind="ExternalInput")
    w2_d = nc.dram_tensor("w2_t", [nlayers, ND, 128, NF, 128], BF16, kind="ExternalInput")
    wh_d = nc.dram_tensor("wh_t", [NV, 128, ND, 512], BF16, kind="ExternalInput")
    prm_d = nc.dram_tensor("prm", [nlayers, 128, NPRM], F32, kind="ExternalInput")
    lnf_d = nc.dram_tensor("lnf", [128, 2 * ND], F32, kind="ExternalInput")
    selA_d = nc.dram_tensor("selA", [6, ND, 128], F32, kind="ExternalInput")
    selB_d = nc.dram_tensor("selB", [6, ND, 128], F32, kind="ExternalInput")
    out_d = nc.dram_tensor("logits", [T2, VP], BF16, kind="ExternalOutput")

    rg = [[0, 1], [2, 3], [4, 5], [6, 7]]

    with tile.TileContext(nc) as tc:
        with tc.tile_pool(name="pers", bufs=1) as pers, \
             tc.tile_pool(name="sb", bufs=1) as sb, \
             tc.tile_pool(name="w", bufs=1) as wp, \
             tc.tile_pool(name="ps", bufs=1, space="PSUM") as ps, \
             tc.tile_pool(name="dram", bufs=1, space="DRAM") as dram:

            # ---------------- persistent tiles ----------------
            h = [pers.tile([128, T2], F32, name=f"h{m}") for m in range(ND)]
            for m in range(ND):
                nc.sync.dma_start(h[m][:], h0_d[:, m, :])
            msk = pers.tile([128, NK, 128], BF16)
            nc.sync.dma_start(msk[:], msk_d[:].rearrange("c p t -> p c t"))

            ones_f = pers.tile([128, 1], F32)
            nc.vector.memset(ones_f[:], 1.0)
            ones_r = pers.tile([128, 1], F32R)          # LN sum lhsT [K=128, M=1]
            nc.vector.tensor_copy(out=ones_r[:], in_=ones_f[:])
            onesM_f = pers.tile([1, 128], F32)
            nc.vector.memset(onesM_f[:], 1.0)
            onesM_r = pers.tile([1, 128], F32R)         # bcast lhsT [K=1, M<=128]
            nc.vector.tensor_copy(out=onesM_r[:], in_=onesM_f[:])

            lnf_sb = pers.tile([128, 2 * ND], F32)
            nc.sync.dma_start(lnf_sb[:], lnf_d[:])
            selA_r = pers.tile([6, ND, 128], F32R)
            selB_r = pers.tile([6, ND, 128], F32R)
            for _sd, _sr in ((selA_d, selA_r), (selB_d, selB_r)):
                sel_f = sb.tile([6, ND, 128], F32, tag="self", bufs=1, name=f"self_{_sd.name}")
                nc.sync.dma_start(sel_f[:], _sd[:])
                nc.vector.tensor_copy(out=_sr[:], in_=sel_f[:])

            import itertools
            _ln_ctr = itertools.count()

            # ---------------- helpers ----------------
            def layer_norm(src, s_ap, b_ap, tag="a"):
                """src: list of ND [128, T2] fp32 tiles -> list of ND bf16 tiles.
                s_ap/b_ap: [128, ND] fp32 scale/bias tiles.
                rstd = exp(-0.5*ln(var+eps)) keeps the serial chain short."""
                # stats borrow the attention po-tag banks (never live at the
                # same time: LN stats need the full residual, which needs all
                # attention outputs)
                s1 = ps.tile([1, T2], F32, tag="po", bufs=2)
                s2 = ps.tile([1, T2], F32, tag="po", bufs=2)
                for m in range(ND):
                    h_r = sb.tile([128, T2], F32R, tag="h_r", bufs=2)
                    nc.vector.tensor_copy(out=h_r[:], in_=src[m][:])
                    hsq = sb.tile([128, T2], F32R, tag="hsq", bufs=2)
                    nc.vector.tensor_mul(out=hsq[:], in0=src[m][:], in1=src[m][:])
                    nc.tensor.matmul(s1[:], ones_r[:], h_r[:], start=(m == 0), stop=(m == ND - 1))
                    nc.tensor.matmul(s2[:], ones_r[:], hsq[:], start=(m == 0), stop=(m == ND - 1))
                # u = (s2 + D*eps) - s1^2/D;  var+eps = u/D (Ln's free affine
                # applies the 1/D scale; Square's applies the 1/D inside)
                ssq = sb.tile([1, T2], F32, tag="lnstat", bufs=6)
                nc.scalar.activation(ssq[:], s1[:], AF.Square, scale=float(D) ** -0.5)
                u = sb.tile([1, T2], F32, tag="lnstat", bufs=6)
                nc.vector.scalar_tensor_tensor(
                    out=u[:], in0=s2[:], scalar=float(D * EPS), in1=ssq[:],
                    op0=OP.add, op1=OP.subtract)
                lnv = sb.tile([1, T2], F32, tag="lnstat", bufs=6)
                nc.scalar.activation(lnv[:], u[:], AF.Ln, scale=1.0 / D)
                rstd = sb.tile([1, T2], F32, tag="lnstat", bufs=6)
                nc.scalar.activation(rstd[:], lnv[:], AF.Exp, scale=-0.5)
                rstd_r = sb.tile([1, T2], F32R, tag="lnstat", bufs=6)
                nc.vector.tensor_copy(out=rstd_r[:], in_=rstd[:])
                mr_r = sb.tile([1, T2], F32R, tag="lnstat", bufs=6)
                nc.vector.scalar_tensor_tensor(
                    out=mr_r[:], in0=s1[:], scalar=1.0 / D, in1=rstd[:],
                    op0=OP.mult, op1=OP.mult)
                a = [sb.tile([128, T2], BF16, tag=f"{tag}{m}", bufs=1, name=f"a_{tag}_{next(_ln_ctr)}_{m}") for m in range(ND)]
                rb = ps.tile([128, T2], F32, tag="mm", bufs=2)
                nc.tensor.matmul(rb[:], onesM_r[:], rstd_r[:], start=True, stop=True)
                mb = ps.tile([128, T2], F32, tag="mm", bufs=2)
                nc.tensor.matmul(mb[:], onesM_r[:], mr_r[:], start=True, stop=True)
                rb_s = sb.tile([128, T2], F32, tag="rb_s", bufs=1)
                nc.vector.tensor_copy(out=rb_s[:], in_=rb[:])
                mb_s = sb.tile([128, T2], F32, tag="mb_s", bufs=1)
                nc.vector.tensor_copy(out=mb_s[:], in_=mb[:])
                for m in range(ND):
                    t1 = sb.tile([128, T2], F32, tag="lnt", bufs=2)
                    nc.vector.scalar_tensor_tensor(
                        out=t1[:], in0=src[m][:], scalar=1.0, in1=rb_s[:],
                        op0=OP.mult, op1=OP.mult)
                    nc.vector.scalar_tensor_tensor(
                        out=t1[:], in0=t1[:], scalar=1.0, in1=mb_s[:],
                        op0=OP.mult, op1=OP.subtract)
                    nc.scalar.activation(
                        a[m][:], t1[:], AF.Identity,
                        bias=b_ap[:, m : m + 1], scale=s_ap[:, m : m + 1])
                return a

            # ---------------- layers ----------------
            for l in range(nlayers):
                prm = sb.tile([128, NPRM], F32, tag="prm", bufs=2)
                nc.sync.dma_start(prm[:], prm_d[l])

                a1 = layer_norm(h, prm[:, P_LN1S : P_LN1S + ND], prm[:, P_LN1B : P_LN1B + ND])

                # V projection (token-major, 65-strided heads + ones col)
                kin_v = dram.tile([T2, 780], BF16, tag="kin_v", bufs=2)
                kout_v = dram.tile([2 * T2, 780], BF16, tag="kout_v", bufs=2)
                wv_sl = wp.tile([128, ND, D], BF16, tag="wv", bufs=1)
                nc.sync.dma_start(wv_sl[:], wv_d[l])
                for t in range(NT):
                    pv1 = ps.tile([128, T2], F32, tag="mm", bufs=2)
                    pv2 = ps.tile([128, 256], F32, tag="mm", bufs=2)
                    for k in range(ND):
                        lhs = a1[k][:, 128 * t : 128 * t + 128]
                        nc.tensor.matmul(pv1[:], lhs, wv_sl[:, k, 0:512], start=(k == 0), stop=(k == ND - 1))
                        nc.tensor.matmul(pv2[:], lhs, wv_sl[:, k, 512:768], start=(k == 0), stop=(k == ND - 1))
                    vc = sb.tile([128, 780], BF16, tag="vc", bufs=2)
                    vch = vc[:].rearrange("p (h e) -> p h e", e=65)
                    nc.vector.tensor_copy(
                        out=vch[:, 0:8, 0:64],
                        in_=pv1[:].rearrange("p (h e) -> p h e", e=64))
                    nc.vector.tensor_copy(
                        out=vch[:, 8:12, 0:64],
                        in_=pv2[:].rearrange("p (h e) -> p h e", e=64))
                    nc.vector.memset(vch[:, :, 64:65], 1.0)
                    nc.sync.dma_start(kin_v[128 * t : 128 * t + 128, :], vc[:])
                nc.gpsimd.collective_compute(
                    "AllGather", OP.bypass,
                    ins=[kin_v[:].opt()], outs=[kout_v[:].opt()], replica_groups=rg)

                # K projection (feature-major) -> two half AGs (heads 0-5 / 6-11)
                kin_k = [dram.tile([D // 2, T2], BF16, tag=f"kin_k{g}", bufs=2, name=f"kin_k{l}_{g}") for g in range(2)]
                kout_k = [dram.tile([D, T2], BF16, tag=f"kout_k{g}", bufs=2, name=f"kout_k{l}_{g}") for g in range(2)]
                for m in range(ND):
                    g, mg = m // 3, m % 3
                    wk_sl = wp.tile([128, ND, 128], BF16, tag="wk", bufs=2)
                    nc.sync.dma_start(wk_sl[:], wk_d[l, m])
                    pk = ps.tile([128, T2], F32, tag="mm", bufs=2)
                    for k in range(ND):
                        nc.tensor.matmul(pk[:], wk_sl[:, k], a1[k][:], start=(k == 0), stop=(k == ND - 1))
                    kc = sb.tile([128, T2], BF16, tag="kc", bufs=2)
                    nc.any.tensor_copy(out=kc[:], in_=pk[:])
                    nc.sync.dma_start(
                        kin_k[g][:].rearrange("(ko ki) t -> ki ko t", ki=128)[:, mg], kc[:])
                    if mg == 2:
                        nc.gpsimd.collective_compute(
                            "AllGather", OP.bypass,
                            ins=[kin_k[g][:].opt()], outs=[kout_k[g][:].opt()],
                            replica_groups=rg)

                # Q projection (feature-major, stays local)
                q = [sb.tile([128, T2], BF16, tag=f"q{m}", bufs=1, name=f"q{l}_{m}") for m in range(ND)]
                for m in range(ND):
                    wq_sl = wp.tile([128, ND, 128], BF16, tag="wq", bufs=2)
                    nc.sync.dma_start(wq_sl[:], wq_d[l, m])
                    pq = ps.tile([128, T2], F32, tag="mm", bufs=2)
                    for k in range(ND):
                        nc.tensor.matmul(pq[:], wq_sl[:, k], a1[k][:], start=(k == 0), stop=(k == ND - 1))
                    nc.any.tensor_copy(out=q[m][:], in_=pq[:])

                # gathered K (feature-major) / V-hat (token-major)
                # kg chunk layout: j = ND*half + ko
                kg = sb.tile([128, 2 * ND, T2], BF16, tag="kg", bufs=1)
                for g in range(2):
                    src_g = kout_k[g][:].rearrange("(hf ko ki) t -> ki hf ko t", ki=128, ko=3)
                    nc.sync.dma_start(kg[:, 3 * g : 3 * g + 3], src_g[:, 0])
                    nc.sync.dma_start(kg[:, ND + 3 * g : ND + 3 * g + 3], src_g[:, 1])
                vg = sb.tile([128, NK, 780], BF16, tag="vg", bufs=1)
                nc.sync.dma_start(vg[:], kout_v[:].rearrange("(to ti) f -> ti to f", ti=128))

                # attention: head pairs (2*hp, 2*hp+1) share feature chunk hp;
                # two denominator groups (pairs 0-2 / 3-5) for overlap
                o = [sb.tile([128, T2], BF16, tag=f"o{m}", bufs=1, name=f"o{l}_{m}") for m in range(ND)]
                dng = [sb.tile([6, T2], F32, tag=f"dn{g}", bufs=1, name=f"dn{l}_{g}") for g in range(2)]
                for hp in range(ND):
                    po2 = [ps.tile([65, T2], F32, tag="po", bufs=2, name=f"po_{l}_{hp}_{j}") for j in range(2)]
                    for c in range(NK):
                        # alternating 128-block split: global key block c lives on
                        # pair-member c%2 at its local block c//2; queries below
                        # local block c//2 never attend to it on either core
                        qlo = 128 * (c // 2)
                        mem, loc = c % 2, c // 2
                        s2j = ps.tile([128, 2, T2], F32, tag="s", bufs=2)
                        for j in range(2):
                            nc.tensor.matmul(
                                s2j[:, j, qlo:],
                                kg[64 * j : 64 * j + 64, ND * mem + hp, 128 * loc : 128 * loc + 128],
                                q[hp][64 * j : 64 * j + 64, qlo:],
                                start=True, stop=True)
                        p_t = sb.tile([128, 2, T2], BF16, tag="p", bufs=5)
                        nc.scalar.activation(p_t[:, :, qlo:], s2j[:, :, qlo:], AF.Exp, scale=SCALE)
                        for j in range(2):
                            # only the diagonal 128-col window ever needs masking
                            nc.vector.tensor_mul(
                                out=p_t[:, j, qlo : qlo + 128],
                                in0=p_t[:, j, qlo : qlo + 128], in1=msk[:, c, :])
                        to = 4 * mem + loc
                        for j in range(2):
                            hi = 2 * hp + j
                            nc.tensor.matmul(
                                po2[j][:, qlo:], vg[:, to, 65 * hi : 65 * hi + 65], p_t[:, j, qlo:],
                                start=(c == 0), stop=(c == NK - 1))
                    g = hp // 3
                    for j in range(2):
                        hi = 2 * hp + j
                        nc.vector.tensor_copy(out=o[hp][64 * j : 64 * j + 64, :], in_=po2[j][0:64, :])
                        dtmp = sb.tile([1, T2], F32, tag="dtmp", bufs=2)
                        nc.vector.tensor_copy(out=dtmp[:], in_=po2[j][64:65, :])
                        nc.sync.dma_start(dng[g][(hi - 6 * g) : (hi - 6 * g) + 1, :], dtmp[:])

                # normalize: 1/denom = exp(-ln(denom)); pair-head broadcast via selector
                for g, sel in ((0, selA_r), (1, selB_r)):
                    nc.scalar.activation(dng[g][:], dng[g][:], AF.Ln)
                    rec_r = sb.tile([6, T2], F32R, tag=f"recr{g}", bufs=1, name=f"recr{l}_{g}")
                    nc.scalar.activation(rec_r[:], dng[g][:], AF.Exp, scale=-1.0)
                    for m in range(3 * g, 3 * g + 3):
                        dnb = ps.tile([128, T2], F32, tag="mm", bufs=2)
                        nc.tensor.matmul(dnb[:], sel[:, m, :], rec_r[:], start=True, stop=True)
                        nc.vector.scalar_tensor_tensor(
                            out=o[m][:], in0=o[m][:], scalar=1.0,
                            in1=dnb[:], op0=OP.mult, op1=OP.mult)

                # output projection + residual
                for m in range(ND):
                    wo_sl = wp.tile([128, ND, 128], BF16, tag="wo", bufs=2)
                    nc.sync.dma_start(wo_sl[:], wo_d[l, m])
                    pw = ps.tile([128, T2], F32, tag="mm", bufs=2)
                    for k in range(ND):
                        nc.tensor.matmul(pw[:], wo_sl[:, k], o[k][:], start=(k == 0), stop=(k == ND - 1))
                    tt = sb.tile([128, T2], F32, tag="res", bufs=2)
                    nc.scalar.activation(tt[:], pw[:], AF.Identity, bias=prm[:, P_BO + m : P_BO + m + 1])
                    nc.vector.tensor_tensor(out=h[m][:], in0=h[m][:], in1=tt[:], op=OP.add)

                # FFN
                a2 = layer_norm(h, prm[:, P_LN2S : P_LN2S + ND], prm[:, P_LN2B : P_LN2B + ND])
                f = [sb.tile([128, T2], BF16, tag=f"f{fc}", bufs=1, name=f"f{l}_{fc}") for fc in range(NF)]
                for fc in range(NF):
                    w1_sl = wp.tile([128, ND, 128], BF16, tag="w1", bufs=3)
                    nc.sync.dma_start(w1_sl[:], w1_d[l, fc])
                    pf = ps.tile([128, T2], F32, tag="mm", bufs=2)
                    for k in range(ND):
                        nc.tensor.matmul(pf[:], w1_sl[:, k], a2[k][:], start=(k == 0), stop=(k == ND - 1))
                    nc.scalar.activation(f[fc][:], pf[:], AF.Relu, bias=prm[:, P_B1 + fc : P_B1 + fc + 1])
                for m in range(ND):
                    w2_sl = wp.tile([128, NF, 128], BF16, tag="w2", bufs=3)
                    nc.sync.dma_start(w2_sl[:], w2_d[l, m])
                    pg = ps.tile([128, T2], F32, tag="mm", bufs=2)
                    for k in range(NF):
                        nc.tensor.matmul(pg[:], w2_sl[:, k], f[k][:], start=(k == 0), stop=(k == NF - 1))
                    tt = sb.tile([128, T2], F32, tag="res", bufs=2)
                    nc.scalar.activation(tt[:], pg[:], AF.Identity, bias=prm[:, P_B2 + m : P_B2 + m + 1])
                    nc.vector.tensor_tensor(out=h[m][:], in0=h[m][:], in1=tt[:], op=OP.add)

            # ---------------- final LN + head ----------------
            hf_t = layer_norm(h, lnf_sb[:, 0:ND], lnf_sb[:, ND : 2 * ND], tag="a")
            for v in range(NV):
                wh_sl = wp.tile([128, ND, 512], BF16, tag="wh", bufs=2)
                nc.sync.dma_start(wh_sl[:], wh_d[v])
                o_dst = out_d[:, 512 * v : 512 * v + 512].rearrange("(to ti) f -> ti to f", ti=128)
                for t in range(NT):
                    pl = ps.tile([128, 512], F32, tag="mm", bufs=2)
                    for k in range(ND):
                        nc.tensor.matmul(
                            pl[:], hf_t[k][:, 128 * t : 128 * t + 128], wh_sl[:, k],
                            start=(k == 0), stop=(k == ND - 1))
                    lg = sb.tile([128, 512], BF16, tag="lg", bufs=4)
                    nc.vector.tensor_copy(out=lg[:], in_=pl[:])
                    nc.sync.dma_start(o_dst[:, t], lg[:])

    import concourse.bacc as _bacc_mod
    _orig_gat = _bacc_mod.get_activation_tables
    _bacc_mod.get_activation_tables = _act_single_set_tables(_orig_gat)
    try:
        nc.compile()
    finally:
        _bacc_mod.get_activation_tables = _orig_gat
    if not nc.is_finalized():
        nc.finalize()
    return nc


def _prep_shared(inputs, nlayers):
    bf = ml_dtypes.bfloat16
    wq, wk, wv, wo = (np.asarray(inputs[k], np.float32) for k in ("wq", "wk", "wv", "wo"))
    w1, w2 = np.asarray(inputs["w1"], np.float32), np.asarray(inputs["w2"], np.float32)
    w_head = np.asarray(inputs["w_head"], np.float32)

    def lhst(w, nm, nk):
        # [L, nk*128, nm*128] -> [L, nm, 128, nk, 128] with [l,m,ki,ko,j] = w[l,128ko+ki,128m+j]
        return np.ascontiguousarray(
            w[:nlayers].reshape(nlayers, nk, 128, nm, 128).transpose(0, 3, 2, 1, 4)).astype(bf)

    d = {}
    d["wq_t"] = lhst(wq, ND, ND)
    d["wk_t"] = lhst(wk, ND, ND)
    d["wo_t"] = lhst(wo, ND, ND)
    d["w1_t"] = lhst(w1, NF, ND)
    d["w2_t"] = lhst(w2, ND, NF)
    d["wv_t"] = np.ascontiguousarray(
        wv[:nlayers].reshape(nlayers, ND, 128, D).transpose(0, 2, 1, 3)).astype(bf)
    whp = np.concatenate([w_head, np.zeros((D, VP - V), np.float32)], axis=1)
    d["wh_t"] = np.ascontiguousarray(
        whp.reshape(ND, 128, NV, 512).transpose(2, 1, 0, 3)).astype(bf)

    prm = np.zeros((nlayers, 128, NPRM), np.float32)

    def chunked(a):  # [L, 768] -> [L, 128, 6]
        return np.asarray(a, np.float32)[:nlayers].reshape(nlayers, -1, 128).transpose(0, 2, 1)

    prm[:, :, P_LN1S : P_LN1S + ND] = chunked(inputs["ln1_s"])
    prm[:, :, P_LN1B : P_LN1B + ND] = chunked(inputs["ln1_b"])
    prm[:, :, P_LN2S : P_LN2S + ND] = chunked(inputs["ln2_s"])
    prm[:, :, P_LN2B : P_LN2B + ND] = chunked(inputs["ln2_b"])
    prm[:, :, P_BO : P_BO + ND] = chunked(inputs["bo"])
    prm[:, :, P_B2 : P_B2 + ND] = chunked(inputs["b2"])
    prm[:, :, P_B1 : P_B1 + NF] = chunked(inputs["b1"])
    d["prm"] = np.ascontiguousarray(prm)

    lnf = np.zeros((128, 2 * ND), np.float32)
    lnf[:, 0:ND] = np.asarray(inputs["lnf_s"], np.float32).reshape(ND, 128).T
    lnf[:, ND : 2 * ND] = np.asarray(inputs["lnf_b"], np.float32).reshape(ND, 128).T
    d["lnf"] = np.ascontiguousarray(lnf)

    selA = np.zeros((6, ND, 128), np.float32)
    selB = np.zeros((6, ND, 128), np.float32)
    for hi in range(H):
        tgt = selA if hi < 6 else selB
        tgt[hi % 6, hi // 2, 64 * (hi % 2) : 64 * (hi % 2) + 64] = 1.0
    d["selA"] = selA
    d["selB"] = selB
    return d


_LAST_RESULTS = None


def kernel(x, tok_emb, pos_emb, wq, wk, wv, wo, bo, ln1_s, ln1_b,
           ln2_s, ln2_b, w1, b1, w2, b2, lnf_s, lnf_b, w_head, b_head,
           nlayers=L):
    global _LAST_RESULTS
    if nlayers not in _BUILT:
        _BUILT[nlayers] = _build(nlayers)
    nc = _BUILT[nlayers]

    inputs = dict(x=x, tok_emb=tok_emb, pos_emb=pos_emb, wq=wq, wk=wk, wv=wv,
                  wo=wo, bo=bo, ln1_s=ln1_s, ln1_b=ln1_b, ln2_s=ln2_s,
                  ln2_b=ln2_b, w1=w1, b1=b1, w2=w2, b2=b2, lnf_s=lnf_s,
                  lnf_b=lnf_b, w_head=w_head, b_head=b_head)
    shared = _prep_shared(inputs, nlayers)

    xi = np.asarray(x).astype(np.int64)
    te = np.asarray(tok_emb, np.float32)
    pe = np.asarray(pos_emb, np.float32)[:T]
    h0 = te[xi] + pe[None, :, :]  # [B, T, D] fp32

    in_maps = []
    for c in range(8):
        b, s = c // 2, c % 2
        # alternating 128-token-block split: core parity s owns global blocks
        # {s, s+2, s+4, s+6}; local block i <-> global block 2i+s
        tok_idx = np.arange(T).reshape(NK, 128)[s::2].reshape(T2)
        hc = np.ascontiguousarray(
            h0[b, tok_idx].T.reshape(ND, 128, T2).transpose(1, 0, 2))
        kk = np.arange(T).reshape(NK, 128)[:, :, None]  # global key pos [c, ki, 1]
        # diagonal window: key chunk c vs this core's local col block c//2
        # (global q block 2*(c//2)+s)
        qq = (128 * (2 * (np.arange(NK) // 2) + s))[:, None, None] + np.arange(128)[None, None, :]
        mc = (kk <= qq).astype(ml_dtypes.bfloat16)
        m = {"h0": hc, "masks": np.ascontiguousarray(mc)}
        m.update(shared)
        in_maps.append(m)

    res = bass_utils.run_bass_kernel_spmd(nc, in_maps, core_ids=list(range(8)))
    _LAST_RESULTS = res

    out = np.empty((B, T, V), np.float32)
    for c in range(8):
        b, s = c // 2, c % 2
        tok_idx = np.arange(T).reshape(NK, 128)[s::2].reshape(T2)
        out[b, tok_idx] = res.results[c]["logits"][:, :V].astype(np.float32)
    bh = np.asarray(b_head, np.float32)
    if np.any(bh):
        out += bh
    return out


if __name__ == "__main__":
    nl = int(os.environ.get("KERNEL_LAYERS", L))
    _build(nl)
    print("build ok", nl)

